# revision 35
# baseline (speedup 1.0000x reference)
"""Trainium2 Bass kernel for CausalSelfAttention with top-2 score filtering.

Reference math (per batch b, head h):
    qkv = x @ w_attn ; split to q,k,v heads [T, D]
    s   = q @ k^T              (top-2 threshold taken over the FULL row,
    kth = 2nd-largest of row    BEFORE the causal mask)
    keep s >= kth, apply causal mask, softmax(s/sqrt(D)), y = attn @ v
    out = concat_heads(y) @ w_proj

Sharding: 32 (b,h) pairs -> 8 cores, 4 heads of one batch each (data
parallel over batch x head-group, column-parallel w_attn, row-parallel
w_proj per the sharding hint).  Each core receives its batch's x
pre-transposed ([C, T], host-side layout prep), computes q^T/k^T ([D, T])
and v ([T, D]), then per 128-query tile: fp32 scores on PE (PSUM -> SBUF),
DVE Max8 for the full-row top-2 values, DVE MaxIndex over the causal
prefix for their positions, and reconstructs each output row as the
softmax-weighted blend of (at most) the two surviving v rows, gathered by
indirect DMA.  Row-parallel c_proj gives a per-core partial [T, C];
partials of the 4 cores of each batch are summed host-side (the final
all-reduce) to form the full output.

All matmuls are true fp32 (4 cyc/row): float32r (TF32-class, ~1.6e-4 rel
err) flips top-2 selections vs the fp32 reference, and even q/k rounded to
f32r shifts scores by ~1e-4, well above typical top-2/3rd gaps.

Rows where no top-2 position is causal are all -inf after masking in the
reference, which yields NaN through softmax; here denom=0 -> 1/0=inf ->
0*inf=NaN reproduces that exactly, and NaN propagates through the PE
transpose + projection to the full output row, matching the reference.
"""

import os
import sys

import numpy as np

for _p in ("/opt/trn_rl_repo", "/root/.axon_site/_ro/trn_rl_repo"):
    if os.path.isdir(_p) and _p not in sys.path:
        sys.path.insert(0, _p)

import concourse.bacc as bacc
import concourse.bass as bass
import concourse.mybir as mybir
import concourse.tile as tile
from concourse import bass_utils

B, T, C, H, D = 2, 2048, 1024, 16, 64
NCORES = 8
HPC = 4  # heads per core
P = 128
NT = T // P  # query tiles per core
F32 = mybir.dt.float32
U32 = mybir.dt.uint32
AF = mybir.ActivationFunctionType
OP = mybir.AluOpType

_CACHE = {}


def _emit(tc, xb, wqk, wv, wpj, qpos, iden, outp, ctx):
    nc = tc.nc

    const = ctx.enter_context(tc.tile_pool(name="const", bufs=1))
    big = ctx.enter_context(tc.tile_pool(name="big", bufs=1))
    dram = ctx.enter_context(tc.tile_pool(name="dram", bufs=1, space="DRAM"))

    ident_sb = const.tile([P, P], F32, tag="ident")
    nc.sync.dma_start(ident_sb[:], iden[:])
    qpos_sb = const.tile([P, NT], F32, tag="qpos")
    nc.sync.dma_start(qpos_sb[:], qpos[:])

    # Persistent SBUF tensors
    xT = [big.tile([P, T], F32, tag=f"xT{g}", name=f"xT{g}") for g in range(8)]
    qkT = [big.tile([P, T], F32, tag=f"qkT{m}", name=f"qkT{m}") for m in range(4)]
    wqk_sb = [
        big.tile([P, 512], F32, tag=f"wqk{g}", name=f"wqk{g}") for g in range(8)
    ]
    wv_sb = [big.tile([P, 256], F32, tag=f"wv{g}", name=f"wv{g}") for g in range(8)]
    wpj_sb = [big.tile([P, C], F32, tag=f"wpj{g}", name=f"wpj{g}") for g in range(2)]
    y_all = big.tile([P, HPC * NT * 64], F32, tag="y_all")
    yT = [big.tile([P, T], F32, tag=f"yT{g}", name=f"yT{g}") for g in range(2)]
    v_dram = [
        dram.tile([T, 64], F32, tag=f"vdram{h}", name=f"vdram{h}") for h in range(HPC)
    ]

    for g in range(8):
        nc.gpsimd.dma_start(wqk_sb[g][:], wqk[g * P : (g + 1) * P, :])
        nc.gpsimd.dma_start(wv_sb[g][:], wv[g * P : (g + 1) * P, :])
    for g in range(2):
        nc.gpsimd.dma_start(wpj_sb[g][:], wpj[g * P : (g + 1) * P, :])

    # ---- load x^T (pre-transposed on host as part of sharding) ---------
    # chunked by 512 columns so the first q/k projection matmuls can start
    # after ~2MB instead of the full 8MB
    for n in range(4):
        for g in range(8):
            eng = (nc.sync, nc.scalar)[g % 2]
            eng.dma_start(
                xT[g][:, n * 512 : (n + 1) * 512],
                xb[g * P : (g + 1) * P, n * 512 : (n + 1) * 512],
            )

    # ---- main phase: q/k projections, v, attention --------------------
    # One PSUM pool so the scheduler can overlap projections with
    # attention: qk(2) + v(2) + scores(4) = 8 banks.
    with tc.tile_pool(name="main_psum", bufs=1, space="PSUM") as mpsum, tc.tile_pool(
        name="ssb_pool", bufs=4
    ) as ssb_pool, tc.tile_pool(name="sm", bufs=4) as sm:
        # q^T / k^T  ([512, T] = wqk^T @ x^T), row-groups m: 0,1=q ; 2,3=k
        for m in range(4):
            for n in range(4):
                ps = mpsum.tile([P, 512], F32, tag="mm", bufs=3, name="qkps")
                for kc in range(8):
                    nc.tensor.matmul(
                        ps[:],
                        wqk_sb[kc][:, m * P : (m + 1) * P],
                        xT[kc][:, n * 512 : (n + 1) * 512],
                        start=(kc == 0),
                        stop=(kc == 7),
                    )
                nc.vector.tensor_copy(qkT[m][:, n * 512 : (n + 1) * 512], ps[:])

        # v natural [T, 256] -> DRAM (gather source)
        for t in range(NT):
            ps = mpsum.tile([P, 1024], F32, tag="vp", bufs=2, name="vps")
            for kc in range(8):
                nc.tensor.matmul(
                    ps[:, 0:256],
                    xT[kc][:, t * P : (t + 1) * P],
                    wv_sb[kc][:],
                    start=(kc == 0),
                    stop=(kc == 7),
                )
            vs = sm.tile([P, 256], F32, tag="v_out", bufs=2)
            nc.scalar.copy(vs[:], ps[:, 0:256])
            for h in range(HPC):
                nc.sync.dma_start(
                    v_dram[h][t * P : (t + 1) * P, :],
                    vs[:, h * 64 : (h + 1) * 64],
                )

        # attention per (query-tile, head), with the projection for each
        # token tile folded in as soon as its 4 heads are transposed
        for t in reversed(range(NT)):
            for h in range(HPC):
                qT = qkT[h // 2][64 * (h % 2) : 64 * (h % 2) + 64, :]
                kT = qkT[2 + h // 2][64 * (h % 2) : 64 * (h % 2) + 64, :]
                scr = ssb_pool.tile([P, T], F32, tag="scr")
                for n in range(4):
                    ps = mpsum.tile([P, 512], F32, tag="mm", bufs=3, name="scps")
                    nc.tensor.matmul(
                        ps[:],
                        qT[:, t * P : (t + 1) * P],
                        kT[:, n * 512 : (n + 1) * 512],
                        start=True,
                        stop=True,
                    )
                    nc.scalar.copy(scr[:, n * 512 : (n + 1) * 512], ps[:])

                top8 = sm.tile([P, 8], F32, tag="top8")
                nc.vector.max(top8[:], scr[:])
                idx8 = sm.tile([P, 8], U32, tag="idx8")
                prefix = (t + 1) * P
                nc.vector.max_index(idx8[:], top8[:], scr[:, 0:prefix])

                # w1 = 1{i1<=q}, w2 = 1{i2<=q} * exp((v2-v1)/8); normalize.
                nv1 = sm.tile([P, 1], F32, tag="nv1")
                nc.vector.tensor_scalar_mul(nv1[:], top8[:, 0:1], -0.125)
                e2 = sm.tile([P, 1], F32, tag="e2")
                nc.scalar.activation(
                    e2[:], top8[:, 1:2], AF.Exp, bias=nv1[:, 0:1], scale=0.125
                )
                if2 = sm.tile([P, 2], F32, tag="if2")
                nc.vector.tensor_copy(if2[:], idx8[:, 0:2])
                valid = sm.tile([P, 2], F32, tag="valid")
                nc.vector.tensor_tensor(
                    valid[:],
                    if2[:],
                    qpos_sb[:, t : t + 1].to_broadcast([P, 2]),
                    op=OP.is_le,
                )
                den = sm.tile([P, 1], F32, tag="den")
                nc.vector.scalar_tensor_tensor(
                    den[:],
                    valid[:, 1:2],
                    e2[:, 0:1],
                    valid[:, 0:1],
                    op0=OP.mult,
                    op1=OP.add,
                )
                rden = sm.tile([P, 1], F32, tag="rden")
                nc.vector.reciprocal(rden[:], den[:])
                w1n = sm.tile([P, 1], F32, tag="w1n")
                nc.vector.tensor_tensor(w1n[:], valid[:, 0:1], rden[:], op=OP.mult)
                w2n = sm.tile([P, 1], F32, tag="w2n")
                nc.vector.scalar_tensor_tensor(
                    w2n[:],
                    valid[:, 1:2],
                    e2[:, 0:1],
                    rden[:, 0:1],
                    op0=OP.mult,
                    op1=OP.mult,
                )
                isafe = sm.tile([P, 2], U32, tag="isafe")
                nc.vector.tensor_scalar_min(isafe[:], idx8[:, 0:2], 2047)

                gg = sm.tile([P, 128], F32, tag="gg")
                nc.gpsimd.indirect_dma_start(
                    out=gg[:, 0:64],
                    out_offset=None,
                    in_=v_dram[h][:],
                    in_offset=bass.IndirectOffsetOnAxis(ap=isafe[:, 0:1], axis=0),
                )
                nc.gpsimd.indirect_dma_start(
                    out=gg[:, 64:128],
                    out_offset=None,
                    in_=v_dram[h][:],
                    in_offset=bass.IndirectOffsetOnAxis(ap=isafe[:, 1:2], axis=0),
                )
                ysl = y_all[:, (h * NT + t) * 64 : (h * NT + t + 1) * 64]
                nc.vector.tensor_scalar(
                    ysl, gg[:, 0:64], w1n[:, 0:1], None, op0=OP.mult
                )
                nc.vector.scalar_tensor_tensor(
                    ysl, gg[:, 64:128], w2n[:, 0:1], ysl, op0=OP.mult, op1=OP.add
                )

                # inline y^T transpose for this (h, t)
                yps = mpsum.tile([P, P], F32, tag="ytr", bufs=1)
                nc.tensor.transpose(yps[0:64, :], ysl, ident_sb[:])
                nc.scalar.copy(
                    yT[h // 2][
                        64 * (h % 2) : 64 * (h % 2) + 64, t * P : (t + 1) * P
                    ],
                    yps[0:64, :],
                )

            # out[t] = y[t] @ w_proj, pipelined inside the attention loop
            pps = mpsum.tile([P, 1024], F32, tag="vp", bufs=2, name="pps")
            for n in range(2):
                for g in range(2):
                    nc.tensor.matmul(
                        pps[:, n * 512 : (n + 1) * 512],
                        yT[g][:, t * P : (t + 1) * P],
                        wpj_sb[g][:, n * 512 : (n + 1) * 512],
                        start=(g == 0),
                        stop=(g == 1),
                    )
            ot = sm.tile([P, C], F32, tag="o_out", bufs=2)
            nc.scalar.copy(ot[:], pps[:])
            nc.sync.dma_start(outp[t * P : (t + 1) * P, :], ot[:])


def _build_nc():
    nc = bacc.Bacc(
        "TRN2",
        target_bir_lowering=False,
        debug=False,
        enable_asserts=False,
        num_devices=NCORES,
    )
    xb = nc.dram_tensor("xb", [C, T], F32, kind="ExternalInput").ap()
    wqk = nc.dram_tensor("wqk", [C, 512], F32, kind="ExternalInput").ap()
    wv = nc.dram_tensor("wv", [C, 256], F32, kind="ExternalInput").ap()
    wpj = nc.dram_tensor("wpj", [256, C], F32, kind="ExternalInput").ap()
    qpos = nc.dram_tensor("qpos", [P, NT], F32, kind="ExternalInput").ap()
    iden = nc.dram_tensor("iden", [P, P], F32, kind="ExternalInput").ap()
    outp = nc.dram_tensor("outp", [T, C], F32, kind="ExternalOutput").ap()
    from contextlib import ExitStack

    with tile.TileContext(nc) as tc:
        with ExitStack() as ctx:
            _emit(tc, xb, wqk, wv, wpj, qpos, iden, outp, ctx)
    nc.finalize()
    return nc


def _in_maps(x, w_attn, w_proj):
    qpos = (np.arange(NT)[None, :] * P + np.arange(P)[:, None]).astype(np.float32)
    iden = np.eye(P, dtype=np.float32)
    maps = []
    for core in range(NCORES):
        b, j = divmod(core, HPC)
        cs = 256 * j
        wqk = np.concatenate(
            [w_attn[:, cs : cs + 256], w_attn[:, C + cs : C + cs + 256]], axis=1
        )
        maps.append(
            {
                "xb": np.ascontiguousarray(x[b].T),
                "wqk": np.ascontiguousarray(wqk),
                "wv": np.ascontiguousarray(w_attn[:, 2 * C + cs : 2 * C + cs + 256]),
                "wpj": np.ascontiguousarray(w_proj[cs : cs + 256, :]),
                "qpos": qpos,
                "iden": iden,
            }
        )
    return maps


def kernel(x, w_attn, w_proj, _trace=False, _trace_kwargs=None):
    x = np.asarray(x, dtype=np.float32)
    w_attn = np.asarray(w_attn, dtype=np.float32)
    w_proj = np.asarray(w_proj, dtype=np.float32)

    if "nc" not in _CACHE:
        _CACHE["nc"] = _build_nc()
    nc = _CACHE["nc"]

    res = bass_utils.run_bass_kernel_spmd(
        nc,
        _in_maps(x, w_attn, w_proj),
        core_ids=list(range(NCORES)),
        trace=_trace,
        **(_trace_kwargs or {}),
    )
    _CACHE["last_results"] = res
    parts = [res.results[i]["outp"] for i in range(NCORES)]
    out = np.stack(
        [
            parts[0] + parts[1] + parts[2] + parts[3],
            parts[4] + parts[5] + parts[6] + parts[7],
        ]
    ).astype(np.float32)
    return out


# revision 36
# speedup vs baseline: 1.0067x; 1.0067x over previous
"""Trainium2 Bass kernel for CausalSelfAttention with top-2 score filtering.

Reference math (per batch b, head h):
    qkv = x @ w_attn ; split to q,k,v heads [T, D]
    s   = q @ k^T              (top-2 threshold taken over the FULL row,
    kth = 2nd-largest of row    BEFORE the causal mask)
    keep s >= kth, apply causal mask, softmax(s/sqrt(D)), y = attn @ v
    out = concat_heads(y) @ w_proj

Sharding: 32 (b,h) pairs -> 8 cores, 4 heads of one batch each (data
parallel over batch x head-group, column-parallel w_attn, row-parallel
w_proj per the sharding hint).  Each core receives its batch's x
pre-transposed ([C, T], host-side layout prep), computes q^T/k^T ([D, T])
and v ([T, D]), then per 128-query tile: fp32 scores on PE (PSUM -> SBUF),
DVE Max8 for the full-row top-2 values, DVE MaxIndex over the causal
prefix for their positions, and reconstructs each output row as the
softmax-weighted blend of (at most) the two surviving v rows, gathered by
indirect DMA.  Row-parallel c_proj gives a per-core partial [T, C];
partials of the 4 cores of each batch are summed host-side (the final
all-reduce) to form the full output.

All matmuls are true fp32 (4 cyc/row): float32r (TF32-class, ~1.6e-4 rel
err) flips top-2 selections vs the fp32 reference, and even q/k rounded to
f32r shifts scores by ~1e-4, well above typical top-2/3rd gaps.

Rows where no top-2 position is causal are all -inf after masking in the
reference, which yields NaN through softmax; here denom=0 -> 1/0=inf ->
0*inf=NaN reproduces that exactly, and NaN propagates through the PE
transpose + projection to the full output row, matching the reference.
"""

import os
import sys

import numpy as np

for _p in ("/opt/trn_rl_repo", "/root/.axon_site/_ro/trn_rl_repo"):
    if os.path.isdir(_p) and _p not in sys.path:
        sys.path.insert(0, _p)

import concourse.bacc as bacc
import concourse.bass as bass
import concourse.mybir as mybir
import concourse.tile as tile
from concourse import bass_utils

B, T, C, H, D = 2, 2048, 1024, 16, 64
NCORES = 8
HPC = 4  # heads per core
P = 128
NT = T // P  # query tiles per core
F32 = mybir.dt.float32
U32 = mybir.dt.uint32
AF = mybir.ActivationFunctionType
OP = mybir.AluOpType

_CACHE = {}


def _emit(tc, xb, wqk, wv, wpj, qpos, iden, outp, ctx):
    nc = tc.nc

    const = ctx.enter_context(tc.tile_pool(name="const", bufs=1))
    big = ctx.enter_context(tc.tile_pool(name="big", bufs=1))
    dram = ctx.enter_context(tc.tile_pool(name="dram", bufs=1, space="DRAM"))

    ident_sb = const.tile([P, P], F32, tag="ident")
    nc.sync.dma_start(ident_sb[:], iden[:])
    qpos_sb = const.tile([P, NT], F32, tag="qpos")
    nc.sync.dma_start(qpos_sb[:], qpos[:])

    # Persistent SBUF tensors
    xT = [big.tile([P, T], F32, tag=f"xT{g}", name=f"xT{g}") for g in range(8)]
    qkT = [big.tile([P, T], F32, tag=f"qkT{m}", name=f"qkT{m}") for m in range(4)]
    wqk_sb = [
        big.tile([P, 512], F32, tag=f"wqk{g}", name=f"wqk{g}") for g in range(8)
    ]
    wv_sb = [big.tile([P, 256], F32, tag=f"wv{g}", name=f"wv{g}") for g in range(8)]
    wpj_sb = [big.tile([P, C], F32, tag=f"wpj{g}", name=f"wpj{g}") for g in range(2)]
    y_all = big.tile([P, HPC * NT * 64], F32, tag="y_all")
    yT = [big.tile([P, T], F32, tag=f"yT{g}", name=f"yT{g}") for g in range(2)]
    v_dram = [
        dram.tile([T, 64], F32, tag=f"vdram{h}", name=f"vdram{h}") for h in range(HPC)
    ]

    for g in range(8):
        nc.gpsimd.dma_start(wqk_sb[g][:], wqk[g * P : (g + 1) * P, :])
        nc.gpsimd.dma_start(wv_sb[g][:], wv[g * P : (g + 1) * P, :])
    for g in range(2):
        nc.gpsimd.dma_start(wpj_sb[g][:], wpj[g * P : (g + 1) * P, :])

    # ---- load x^T (pre-transposed on host as part of sharding) ---------
    # chunked by 512 columns so the first q/k projection matmuls can start
    # after ~2MB instead of the full 8MB
    for n in range(4):
        for g in range(8):
            eng = (nc.sync, nc.scalar)[g % 2]
            eng.dma_start(
                xT[g][:, n * 512 : (n + 1) * 512],
                xb[g * P : (g + 1) * P, n * 512 : (n + 1) * 512],
            )

    # ---- main phase: q/k projections, v, attention --------------------
    # One PSUM pool so the scheduler can overlap projections with
    # attention: qk(2) + v(2) + scores(4) = 8 banks.
    with tc.tile_pool(name="main_psum", bufs=1, space="PSUM") as mpsum, tc.tile_pool(
        name="ssb_pool", bufs=4
    ) as ssb_pool, tc.tile_pool(name="sm", bufs=4) as sm:
        # q^T / k^T  ([512, T] = wqk^T @ x^T), row-groups m: 0,1=q ; 2,3=k
        for m in range(4):
            for n in range(4):
                ps = mpsum.tile([P, 512], F32, tag="mm", bufs=3, name="qkps")
                for kc in range(8):
                    nc.tensor.matmul(
                        ps[:],
                        wqk_sb[kc][:, m * P : (m + 1) * P],
                        xT[kc][:, n * 512 : (n + 1) * 512],
                        start=(kc == 0),
                        stop=(kc == 7),
                    )
                nc.vector.tensor_copy(qkT[m][:, n * 512 : (n + 1) * 512], ps[:])

        # v natural [T, 256] -> DRAM (gather source)
        for t in range(NT):
            ps = mpsum.tile([P, 1024], F32, tag="vp", bufs=2, name="vps")
            for kc in range(8):
                nc.tensor.matmul(
                    ps[:, 0:256],
                    xT[kc][:, t * P : (t + 1) * P],
                    wv_sb[kc][:],
                    start=(kc == 0),
                    stop=(kc == 7),
                )
            vs = sm.tile([P, 256], F32, tag="v_out", bufs=2)
            nc.scalar.copy(vs[:], ps[:, 0:256])
            for h in range(HPC):
                nc.sync.dma_start(
                    v_dram[h][t * P : (t + 1) * P, :],
                    vs[:, h * 64 : (h + 1) * 64],
                )

        # pre-zero the gather-slot pool so rows skipped by the bounds
        # check (invalid index 0xFFFFFFFF) never expose uninitialized SBUF
        for _ in range(4):
            gz = sm.tile([P, 128], F32, tag="gg", name="gz")
            nc.vector.memset(gz[:], 0.0)

        # attention per (query-tile, head), with the projection for each
        # token tile folded in as soon as its 4 heads are transposed
        for t in reversed(range(NT)):
            for h in range(HPC):
                qT = qkT[h // 2][64 * (h % 2) : 64 * (h % 2) + 64, :]
                kT = qkT[2 + h // 2][64 * (h % 2) : 64 * (h % 2) + 64, :]
                scr = ssb_pool.tile([P, T], F32, tag="scr")
                for n in range(4):
                    ps = mpsum.tile([P, 512], F32, tag="mm", bufs=3, name="scps")
                    nc.tensor.matmul(
                        ps[:],
                        qT[:, t * P : (t + 1) * P],
                        kT[:, n * 512 : (n + 1) * 512],
                        start=True,
                        stop=True,
                    )
                    nc.scalar.copy(scr[:, n * 512 : (n + 1) * 512], ps[:])

                top8 = sm.tile([P, 8], F32, tag="top8")
                nc.vector.max(top8[:], scr[:])
                idx8 = sm.tile([P, 8], U32, tag="idx8")
                prefix = (t + 1) * P
                nc.vector.max_index(idx8[:], top8[:], scr[:, 0:prefix])

                # w1 = 1{i1<=q}, w2 = 1{i2<=q} * exp((v2-v1)/8); normalize.
                nv1 = sm.tile([P, 1], F32, tag="nv1")
                nc.vector.tensor_scalar_mul(nv1[:], top8[:, 0:1], -0.125)
                e2 = sm.tile([P, 1], F32, tag="e2")
                nc.scalar.activation(
                    e2[:], top8[:, 1:2], AF.Exp, bias=nv1[:, 0:1], scale=0.125
                )
                if2 = sm.tile([P, 2], F32, tag="if2")
                nc.vector.tensor_copy(if2[:], idx8[:, 0:2])
                valid = sm.tile([P, 2], F32, tag="valid")
                nc.vector.tensor_tensor(
                    valid[:],
                    if2[:],
                    qpos_sb[:, t : t + 1].to_broadcast([P, 2]),
                    op=OP.is_le,
                )
                den = sm.tile([P, 1], F32, tag="den")
                nc.vector.scalar_tensor_tensor(
                    den[:],
                    valid[:, 1:2],
                    e2[:, 0:1],
                    valid[:, 0:1],
                    op0=OP.mult,
                    op1=OP.add,
                )
                rden = sm.tile([P, 1], F32, tag="rden")
                nc.vector.reciprocal(rden[:], den[:])
                w1n = sm.tile([P, 1], F32, tag="w1n")
                nc.vector.tensor_tensor(w1n[:], valid[:, 0:1], rden[:], op=OP.mult)
                w2n = sm.tile([P, 1], F32, tag="w2n")
                nc.vector.scalar_tensor_tensor(
                    w2n[:],
                    valid[:, 1:2],
                    e2[:, 0:1],
                    rden[:, 0:1],
                    op0=OP.mult,
                    op1=OP.mult,
                )
                gg = sm.tile([P, 128], F32, tag="gg")
                nc.gpsimd.indirect_dma_start(
                    out=gg[:, 0:64],
                    out_offset=None,
                    in_=v_dram[h][:],
                    in_offset=bass.IndirectOffsetOnAxis(ap=idx8[:, 0:1], axis=0),
                    bounds_check=2047,
                    oob_is_err=False,
                )
                nc.gpsimd.indirect_dma_start(
                    out=gg[:, 64:128],
                    out_offset=None,
                    in_=v_dram[h][:],
                    in_offset=bass.IndirectOffsetOnAxis(ap=idx8[:, 1:2], axis=0),
                    bounds_check=2047,
                    oob_is_err=False,
                )
                ysl = y_all[:, (h * NT + t) * 64 : (h * NT + t + 1) * 64]
                nc.vector.tensor_scalar(
                    ysl, gg[:, 0:64], w1n[:, 0:1], None, op0=OP.mult
                )
                nc.vector.scalar_tensor_tensor(
                    ysl, gg[:, 64:128], w2n[:, 0:1], ysl, op0=OP.mult, op1=OP.add
                )

                # inline y^T transpose for this (h, t)
                yps = mpsum.tile([P, P], F32, tag="ytr", bufs=1)
                nc.tensor.transpose(yps[0:64, :], ysl, ident_sb[:])
                nc.scalar.copy(
                    yT[h // 2][
                        64 * (h % 2) : 64 * (h % 2) + 64, t * P : (t + 1) * P
                    ],
                    yps[0:64, :],
                )

            # out[t] = y[t] @ w_proj, pipelined inside the attention loop
            pps = mpsum.tile([P, 1024], F32, tag="vp", bufs=2, name="pps")
            for n in range(2):
                for g in range(2):
                    nc.tensor.matmul(
                        pps[:, n * 512 : (n + 1) * 512],
                        yT[g][:, t * P : (t + 1) * P],
                        wpj_sb[g][:, n * 512 : (n + 1) * 512],
                        start=(g == 0),
                        stop=(g == 1),
                    )
            ot = sm.tile([P, C], F32, tag="o_out", bufs=2)
            nc.scalar.copy(ot[:], pps[:])
            nc.sync.dma_start(outp[t * P : (t + 1) * P, :], ot[:])


def _build_nc():
    nc = bacc.Bacc(
        "TRN2",
        target_bir_lowering=False,
        debug=False,
        enable_asserts=False,
        num_devices=NCORES,
    )
    xb = nc.dram_tensor("xb", [C, T], F32, kind="ExternalInput").ap()
    wqk = nc.dram_tensor("wqk", [C, 512], F32, kind="ExternalInput").ap()
    wv = nc.dram_tensor("wv", [C, 256], F32, kind="ExternalInput").ap()
    wpj = nc.dram_tensor("wpj", [256, C], F32, kind="ExternalInput").ap()
    qpos = nc.dram_tensor("qpos", [P, NT], F32, kind="ExternalInput").ap()
    iden = nc.dram_tensor("iden", [P, P], F32, kind="ExternalInput").ap()
    outp = nc.dram_tensor("outp", [T, C], F32, kind="ExternalOutput").ap()
    from contextlib import ExitStack

    with tile.TileContext(nc) as tc:
        with ExitStack() as ctx:
            _emit(tc, xb, wqk, wv, wpj, qpos, iden, outp, ctx)
    nc.finalize()
    return nc


def _in_maps(x, w_attn, w_proj):
    qpos = (np.arange(NT)[None, :] * P + np.arange(P)[:, None]).astype(np.float32)
    iden = np.eye(P, dtype=np.float32)
    maps = []
    for core in range(NCORES):
        b, j = divmod(core, HPC)
        cs = 256 * j
        wqk = np.concatenate(
            [w_attn[:, cs : cs + 256], w_attn[:, C + cs : C + cs + 256]], axis=1
        )
        maps.append(
            {
                "xb": np.ascontiguousarray(x[b].T),
                "wqk": np.ascontiguousarray(wqk),
                "wv": np.ascontiguousarray(w_attn[:, 2 * C + cs : 2 * C + cs + 256]),
                "wpj": np.ascontiguousarray(w_proj[cs : cs + 256, :]),
                "qpos": qpos,
                "iden": iden,
            }
        )
    return maps


def kernel(x, w_attn, w_proj, _trace=False, _trace_kwargs=None):
    x = np.asarray(x, dtype=np.float32)
    w_attn = np.asarray(w_attn, dtype=np.float32)
    w_proj = np.asarray(w_proj, dtype=np.float32)

    if "nc" not in _CACHE:
        _CACHE["nc"] = _build_nc()
    nc = _CACHE["nc"]

    res = bass_utils.run_bass_kernel_spmd(
        nc,
        _in_maps(x, w_attn, w_proj),
        core_ids=list(range(NCORES)),
        trace=_trace,
        **(_trace_kwargs or {}),
    )
    _CACHE["last_results"] = res
    parts = [res.results[i]["outp"] for i in range(NCORES)]
    out = np.stack(
        [
            parts[0] + parts[1] + parts[2] + parts[3],
            parts[4] + parts[5] + parts[6] + parts[7],
        ]
    ).astype(np.float32)
    return out
